# revision 14
# baseline (speedup 1.0000x reference)
"""Causal multi-head attention (16 heads, head_dim 128, QK-RMSNorm + RoPE)
distributed over 8 Trainium2 NeuronCores.

Sharding: tensor-parallel over heads (4 heads / core) x data-parallel over
batch (B=2): core c handles batch b=c//4, head group g=c%4 (inner columns
512*g : 512*(g+1)).

Per-core device program (SPMD, identical on all cores):
  P1  Q/K/V projections in natural layout [n, 512] (lhsT = x^T tiles,
      rhs = W column slices), fp32r matmuls.  Q/K chunks bounce via DRAM
      (SBUF pressure), V stays resident.  Per-row sum-of-squares for the
      QK RMSNorm is accumulated from PSUM with ACT Square+accum.
  P2  AllReduce (groups [[0..3],[4..7]]) of the partial sumsq -> rms.
  P3  RoPE in natural layout (free-dim half rotation), rms_q pre-applied to
      Q; then PE-transpose per head -> qT/kT [dh=128, n] layout.
  P4  Causal attention per (q-tile 512, head): S^T = kT^T @ qT chunks
      [k=128, q=512]; exp on ACT with per-partition scale folding
      1/(rms_k*sqrt(dh)); causal handled by skipping invisible k-chunks,
      memset + one triangular-mask multiply on diagonal blocks;
      PV accumulates O^T [dh, q] in PSUM; softmax denominator l via a
      ones-column matmul accumulated in PSUM, applied as 1/l through a
      gpsimd partition-broadcast.
  P6  Output projection with Wo rows local to the core (O^T as moving
      operand, Wo stationary) -> partial out^T; ReduceScatter(add) within
      the group hands each core the finished out^T for its q block.

Host: slices/transposes inputs, builds RoPE tables, gathers per-core
out^T blocks into the full [2, 2048, 2048] output.
"""

import numpy as np

B = 2
N = 2048          # sequence length
D = 2048          # model dim
H = 16            # total heads
DH = 128          # head dim
HPC = 4           # heads per core
IPC = HPC * DH    # inner dims per core = 512
NCH = N // 128    # 16 partition chunks of the sequence
KD = D // 128     # 16 contraction chunks of the model dim
NQT = N // 512    # 4 q tiles of 512
ROPE_BASE = 50000.0
EPS = 1e-6
SCALE = 1.0 / np.sqrt(DH)
N_CORES = 8
GROUPS = [[0, 1, 2, 3], [4, 5, 6, 7]]

_cache = {}


def _build_program(apply_qn: bool):
    import concourse.bass as bass
    import concourse.mybir as mybir
    import concourse.tile as tile
    from concourse import bacc

    f32 = mybir.dt.float32
    f32r = mybir.dt.float32r
    AF = mybir.ActivationFunctionType
    Alu = mybir.AluOpType

    nc = bacc.Bacc("TRN2", target_bir_lowering=False, debug=False,
                   num_devices=N_CORES)

    # ---- I/O ----
    xT = nc.dram_tensor("xT", [D, N], f32r, kind="ExternalInput").ap()
    wq = nc.dram_tensor("wq", [D, IPC], f32r, kind="ExternalInput").ap()
    wk = nc.dram_tensor("wk", [D, IPC], f32r, kind="ExternalInput").ap()
    wv = nc.dram_tensor("wv", [D, IPC], f32r, kind="ExternalInput").ap()
    wo = nc.dram_tensor("wo", [IPC, D], f32r, kind="ExternalInput").ap()
    qn = nc.dram_tensor("qn", [1, IPC], f32, kind="ExternalInput").ap()
    kn = nc.dram_tensor("kn", [1, IPC], f32, kind="ExternalInput").ap()
    cos_d = nc.dram_tensor("cos", [N, DH], f32, kind="ExternalInput").ap()
    sin_d = nc.dram_tensor("sin_s", [N, DH], f32, kind="ExternalInput").ap()
    tri_d = nc.dram_tensor("tri", [128, 128], f32r, kind="ExternalInput").ap()
    idn_d = nc.dram_tensor("idn", [128, 128], f32, kind="ExternalInput").ap()
    ones_d = nc.dram_tensor("ones_col", [128, 2], f32r, kind="ExternalInput").ap()
    outT = nc.dram_tensor("outT", [D, 512], f32, kind="ExternalOutput").ap()

    xT_r = xT.rearrange("(ko p) n -> p ko n", p=128)      # [128, KD, N]
    wq_r = wq.rearrange("(ko p) i -> p ko i", p=128)      # [128, KD, IPC]
    wk_r = wk.rearrange("(ko p) i -> p ko i", p=128)
    wv_r = wv.rearrange("(ko p) i -> p ko i", p=128)
    wo_r = wo.rearrange("(io p) m -> p io m", p=128)      # [128, 4, D]
    cos_r = cos_d.rearrange("(c p) d -> p c d", p=128)    # [128, NCH, DH]
    sin_r = sin_d.rearrange("(c p) d -> p c d", p=128)

    with tile.TileContext(nc) as tc:
        with (
            tc.tile_pool(name="dram", bufs=1, space="DRAM") as dram,
            tc.tile_pool(name="const", bufs=1) as const,
            tc.tile_pool(name="sb", bufs=1) as sb,
        ):
            # ---------- constants ----------
            tri = const.tile([128, 128], f32r, tag="tri", name="tri_sb")
            nc.sync.dma_start(tri[:], tri_d)
            idn = const.tile([128, 128], f32, tag="idn", name="idn_sb")
            nc.sync.dma_start(idn[:], idn_d)
            ones_col = const.tile([128, 2], f32r, tag="ones", name="ones_sb")
            nc.sync.dma_start(ones_col[:], ones_d)
            eps_t = const.tile([128, 1], f32, tag="eps", name="eps_t")
            nc.gpsimd.memset(eps_t[:], EPS)
            if apply_qn:
                qn_b = const.tile([128, IPC], f32, tag="qn_b", name="qn_b")
                nc.sync.dma_start(qn_b[:], qn.to_broadcast((128, IPC)))
                kn_b = const.tile([128, IPC], f32, tag="kn_b", name="kn_b")
                nc.sync.dma_start(kn_b[:], kn.to_broadcast((128, IPC)))

            # DRAM bounce for q/k natural chunks (per-chunk tiles so P3
            # reads only depend on their own chunk's write, letting K-side
            # rope/transpose overlap P1's tail and the rms AllReduce)
            qnat_d = [dram.tile([128, IPC], f32, name=f"qnat_d{i}")
                      for i in range(NCH)]
            knat_d = [dram.tile([128, IPC], f32, name=f"knat_d{i}")
                      for i in range(NCH)]
            ssq_in = dram.tile([128, 32], f32, name="ssq_in")
            ssq_out = dram.tile([128, 32], f32, name="ssq_out")
            rs_in = dram.tile([NQT, D, 512], f32, name="rs_in")
            rs_out = dram.tile([D, 512], f32, name="rs_out")

            ssq = sb.tile([128, 32], f32, tag="ssq", name="ssq")

            v_tiles = []

            # ================= P1: QKV projections (natural) =================
            with (
                tc.tile_pool(name="w_pool", bufs=1) as wpool,
                tc.tile_pool(name="p1", bufs=2) as p1,
                tc.tile_pool(name="psA", bufs=6, space="PSUM") as psA,
            ):
                wq_sb = wpool.tile([128, KD, IPC], f32r, tag="wq", name="wq_sb")
                nc.sync.dma_start(wq_sb[:], wq_r)
                wk_sb = wpool.tile([128, KD, IPC], f32r, tag="wk", name="wk_sb")
                nc.sync.dma_start(wk_sb[:], wk_r)
                wv_sb = wpool.tile([128, KD, IPC], f32r, tag="wv", name="wv_sb")
                nc.sync.dma_start(wv_sb[:], wv_r)

                for nci in range(NCH):
                    xcol = p1.tile([128, KD, 128], f32r, tag="xcol",
                                   name=f"xcol{nci}", bufs=2)
                    nc.gpsimd.dma_start(xcol[:], xT_r[:, :, nci * 128:(nci + 1) * 128])

                    ps_q = psA.tile([128, 512], f32, tag="p1", bufs=6,
                                    name=f"psq{nci}")
                    ps_k = psA.tile([128, 512], f32, tag="p1", bufs=6,
                                    name=f"psk{nci}")
                    ps_v = psA.tile([128, 512], f32, tag="p1", bufs=6,
                                    name=f"psv{nci}")
                    for dk in range(KD):
                        lhs = xcol[:, dk, :]
                        st = dk == 0
                        sp = dk == KD - 1
                        nc.tensor.matmul(ps_q[:], lhs, wq_sb[:, dk, :],
                                         start=st, stop=sp)
                        nc.tensor.matmul(ps_k[:], lhs, wk_sb[:, dk, :],
                                         start=st, stop=sp)
                        nc.tensor.matmul(ps_v[:], lhs, wv_sb[:, dk, :],
                                         start=st, stop=sp)

                    # sum-of-squares for RMS (from raw psum, pre-qn)
                    sq_scr = p1.tile([128, 512], f32, tag="sq_scr",
                                     name=f"sqs{nci}", bufs=2)
                    nc.scalar.activation(sq_scr[:], ps_q[:], AF.Square,
                                         accum_out=ssq[:, nci:nci + 1])
                    sq_scr2 = p1.tile([128, 512], f32, tag="sq_scr",
                                      name=f"sqs2_{nci}", bufs=2)
                    nc.scalar.activation(sq_scr2[:], ps_k[:], AF.Square,
                                         accum_out=ssq[:, 16 + nci:17 + nci])

                    # evictions
                    qev = p1.tile([128, 512], f32, tag="qev",
                                  name=f"qev{nci}", bufs=3)
                    kev = p1.tile([128, 512], f32, tag="kev",
                                  name=f"kev{nci}", bufs=3)
                    if apply_qn:
                        nc.vector.tensor_mul(qev[:], ps_q[:], qn_b[:])
                        nc.vector.tensor_mul(kev[:], ps_k[:], kn_b[:])
                    else:
                        nc.vector.tensor_copy(qev[:], ps_q[:])
                        nc.vector.tensor_copy(kev[:], ps_k[:])
                    nc.sync.dma_start(qnat_d[nci][:], qev[:])
                    nc.gpsimd.dma_start(knat_d[nci][:], kev[:])

                    v_t = sb.tile([128, 512], f32r, tag=f"v{nci}", name=f"v{nci}")
                    nc.vector.tensor_copy(v_t[:], ps_v[:])
                    v_tiles.append(v_t)

            # ================= P2: RMS allreduce =================
            nc.sync.dma_start(ssq_in[:], ssq[:])
            nc.gpsimd.collective_compute(
                "AllReduce", Alu.add, replica_groups=GROUPS,
                ins=[ssq_in.opt()], outs=[ssq_out.opt()],
            )
            ssq_all = sb.tile([128, 32], f32, tag="ssq_all", name="ssq_all")
            nc.sync.dma_start(ssq_all[:], ssq_out[:])
            rms = sb.tile([128, 32], f32, tag="rms", name="rms")
            nc.scalar.activation(rms[:], ssq_all[:], AF.Sqrt,
                                 scale=1.0 / D, bias=eps_t[:])
            rr = sb.tile([128, 32], f32, tag="rr", name="rr")
            nc.vector.reciprocal(rr[:], rms[:])
            # rrq = rr[:, 0:16] ; rrk_scaled = rr[:,16:32] * (1/sqrt(DH))
            rrk_s = sb.tile([128, 16], f32, tag="rrk_s", name="rrk_s")
            nc.vector.tensor_scalar_mul(rrk_s[:], rr[:, 16:32], SCALE)

            # ================= P3: RoPE + transpose =================
            late = tc.alloc_tile_pool(name="late", bufs=1)
            qT = [late.tile([128, N], f32r, tag=f"qT{h}", name=f"qT{h}")
                  for h in range(HPC)]
            kT = [late.tile([128, N], f32r, tag=f"kT{h}", name=f"kT{h}")
                  for h in range(HPC)]

            with (
                tc.tile_pool(name="p3", bufs=4) as p3,
                tc.tile_pool(name="psT", bufs=1, space="PSUM") as psT,
            ):
                cos_sb = p3.tile([128, NCH, DH], f32, tag="cos", name="cos_sb",
                                 bufs=1)
                nc.sync.dma_start(cos_sb[:], cos_r)
                sin_sb = p3.tile([128, NCH, DH], f32, tag="sin", name="sin_sb",
                                 bufs=1)
                nc.sync.dma_start(sin_sb[:], sin_r)

                for nci in range(NCH):
                    for which, nat_d, dstT in (("q", qnat_d, qT), ("k", knat_d, kT)):
                        ch = p3.tile([128, HPC, DH], f32, tag=f"{which}ch",
                                     name=f"{which}ch{nci}")
                        dma_eng = nc.sync if which == "q" else nc.gpsimd
                        dma_eng.dma_start(
                            ch[:], nat_d[nci][:]
                            .rearrange("p (h d) -> p h d", h=HPC))
                        if which == "q":
                            # pre-scale by 1/rms_q (per-row)
                            chs = p3.tile([128, HPC, DH], f32, tag="qchs",
                                          name=f"qchs{nci}")
                            nc.vector.tensor_scalar_mul(chs[:], ch[:],
                                                        rr[:, nci:nci + 1])
                            ch = chs
                        cos_bc = cos_sb[:, nci:nci + 1, :].to_broadcast(
                            (128, HPC, DH))
                        t1 = p3.tile([128, HPC, DH], f32, tag="t1",
                                     name=f"t1_{which}{nci}")
                        nc.vector.tensor_mul(t1[:], ch[:], cos_bc)
                        t2 = p3.tile([128, HPC, DH], f32, tag="t2",
                                     name=f"t2_{which}{nci}")
                        nc.vector.tensor_mul(
                            t2[:, :, 0:64], ch[:, :, 64:128],
                            sin_sb[:, nci:nci + 1, 0:64].to_broadcast((128, HPC, 64)))
                        nc.vector.tensor_mul(
                            t2[:, :, 64:128], ch[:, :, 0:64],
                            sin_sb[:, nci:nci + 1, 64:128].to_broadcast((128, HPC, 64)))
                        rp = p3.tile([128, HPC, DH], f32, tag="rp",
                                     name=f"rp_{which}{nci}")
                        nc.vector.tensor_add(rp[:], t1[:], t2[:])
                        for h in range(HPC):
                            ps_t = psT.tile([128, 128], f32, tag="ps_t",
                                            bufs=2, name=f"pst_{which}{nci}_{h}")
                            nc.tensor.transpose(ps_t[:], rp[:, h, :], idn[:])
                            nc.scalar.copy(
                                dstT[h][:, nci * 128:(nci + 1) * 128], ps_t[:])

            # ================= P4 + P6 =================
            wo_sb = late.tile([128, HPC, D], f32r, tag="wo_sb", name="wo_sb")
            nc.sync.dma_start(wo_sb[:], wo_r)
            with (
                tc.tile_pool(name="p4", bufs=1) as p4,
                tc.tile_pool(name="psB", bufs=1, space="PSUM") as psB,
            ):
                for qt in range(NQT):
                    o_tiles = []
                    for h in range(HPC):
                        n_kc = 4 * (qt + 1)
                        ps_o = psB.tile([128, 512], f32, tag="ps_o", bufs=2,
                                        name=f"pso{qt}_{h}")
                        ps_l = psB.tile([2, 512], f32, tag="ps_l", bufs=1,
                                        name=f"psl{qt}_{h}")
                        for kc in range(n_kc):
                            ps_s = psB.tile([128, 512], f32, tag="ps_a", bufs=4,
                                            name=f"pss{qt}_{h}_{kc}")
                            nc.tensor.matmul(
                                ps_s[:],
                                kT[h][:, kc * 128:(kc + 1) * 128],
                                qT[h][:, qt * 512:(qt + 1) * 512],
                                start=True, stop=True)
                            pT = p4.tile([128, 512], f32r, tag="pT",
                                         name=f"pT{qt}_{h}_{kc}", bufs=4)
                            j = kc - 4 * qt
                            if j < 0:
                                # fully visible chunk
                                nc.scalar.activation(pT[:], ps_s[:], AF.Exp,
                                                     scale=rrk_s[:, kc:kc + 1])
                                j = 0
                            else:
                                # diagonal block-column: q cols < 128*j are
                                # invisible; never compute or read them.
                                nc.scalar.activation(
                                    pT[:, j * 128:], ps_s[:, j * 128:], AF.Exp,
                                    scale=rrk_s[:, kc:kc + 1])
                                nc.vector.tensor_mul(
                                    pT[:, j * 128:(j + 1) * 128],
                                    pT[:, j * 128:(j + 1) * 128], tri[:])
                            st = kc == 0
                            sp = kc == n_kc - 1
                            nc.tensor.matmul(ps_o[:, j * 128:],
                                             v_tiles[kc][:, h * 128:(h + 1) * 128],
                                             pT[:, j * 128:],
                                             start=st, stop=sp)
                            nc.tensor.matmul(ps_l[:, j * 128:], ones_col[:],
                                             pT[:, j * 128:],
                                             start=st, stop=sp)
                        # 1/l, broadcast across partitions, apply at evict
                        rl = p4.tile([1, 512], f32, tag="rl",
                                     name=f"rl{qt}_{h}", bufs=2)
                        nc.vector.reciprocal(rl[:], ps_l[0:1, :])
                        rlb = p4.tile([128, 512], f32, tag="rlb",
                                      name=f"rlb{qt}_{h}", bufs=2)
                        nc.gpsimd.partition_broadcast(rlb[:], rl[:])
                        o_t = p4.tile([128, 512], f32r, tag="o_t",
                                      name=f"o{qt}_{h}", bufs=6)
                        nc.vector.tensor_mul(o_t[:], ps_o[:], rlb[:])
                        o_tiles.append(o_t)

                    # P6 for this q tile: Wo-stationary partial out^T
                    for dm in range(KD):
                        ps_f = psB.tile([128, 512], f32, tag="ps_a", bufs=4,
                                        name=f"psf{qt}_{dm}")
                        for ic in range(HPC):
                            nc.tensor.matmul(
                                ps_f[:],
                                wo_sb[:, ic, dm * 128:(dm + 1) * 128],
                                o_tiles[ic][:],
                                start=(ic == 0), stop=(ic == HPC - 1))
                        fev = p4.tile([128, 512], f32, tag="fev",
                                      name=f"fev{qt}_{dm}", bufs=4)
                        nc.any.tensor_copy(out=fev[:], in_=ps_f[:])
                        eng = nc.sync if dm % 2 == 0 else nc.gpsimd
                        eng.dma_start(
                            rs_in[qt, dm * 128:(dm + 1) * 128, :], fev[:])

            late.release()

            # ================= single ReduceScatter + output =================
            nc.gpsimd.collective_compute(
                "ReduceScatter", Alu.add, replica_groups=GROUPS,
                ins=[rs_in.opt()], outs=[rs_out.opt()],
            )
            nc.sync.dma_start(outT[:], rs_out[:])

    nc.compile()
    return nc


def _get_program(apply_qn: bool):
    key = ("prog", apply_qn)
    if key not in _cache:
        _cache[key] = _build_program(apply_qn)
    return _cache[key]


def _rope_tables():
    inv_freq = (1.0 / (ROPE_BASE ** (np.arange(0, DH, 2, dtype=np.float32) / DH))
                ).astype(np.float32)
    t = np.arange(N, dtype=np.float32)
    freqs = np.outer(t, inv_freq).astype(np.float32)       # [N, DH/2]
    emb = np.concatenate([freqs, freqs], axis=-1)          # [N, DH]
    cos = np.cos(emb).astype(np.float32)
    sin = np.sin(emb).astype(np.float32)
    sin_s = sin.copy()
    sin_s[:, 0:DH // 2] *= -1.0
    return cos, sin_s


def make_in_maps(x, Wq, Wk, Wv, Wo, qn_w, kn_w):
    cos, sin_s = _rope_tables()
    tri = np.triu(np.ones((128, 128), dtype=np.float32))
    idn = np.eye(128, dtype=np.float32)
    ones_col = np.ones((128, 2), dtype=np.float32)
    in_maps = []
    for c in range(N_CORES):
        b, g = c // 4, c % 4
        sl = slice(g * IPC, (g + 1) * IPC)
        in_maps.append({
            "xT": np.ascontiguousarray(x[b].T),
            "wq": np.ascontiguousarray(Wq[:, sl]),
            "wk": np.ascontiguousarray(Wk[:, sl]),
            "wv": np.ascontiguousarray(Wv[:, sl]),
            "wo": np.ascontiguousarray(Wo[sl, :]),
            "qn": np.ascontiguousarray(qn_w[sl]).reshape(1, IPC),
            "kn": np.ascontiguousarray(kn_w[sl]).reshape(1, IPC),
            "cos": cos, "sin_s": sin_s,
            "tri": tri, "idn": idn, "ones_col": ones_col,
        })
    return in_maps


def assemble_output(results):
    out = np.empty((B, N, D), dtype=np.float32)
    for c in range(N_CORES):
        b, j = c // 4, c % 4
        out[b, j * 512:(j + 1) * 512, :] = results[c]["outT"].T
    return out


def _get_runner(apply_qn: bool):
    """Build (once) a cached jitted PJRT runner for the 8-core program.

    Mirrors concourse.bass2jax.run_bass_via_pjrt, but keeps the jitted
    shard_map callable so repeat kernel() calls don't re-trace/re-compile.
    """
    key = ("runner", apply_qn)
    if key in _cache:
        return _cache[key]

    import jax
    from jax.sharding import Mesh, PartitionSpec
    try:
        from jax.experimental.shard_map import shard_map
    except ImportError:
        from jax.shard_map import shard_map
    import concourse.mybir as mybir
    from concourse.bass2jax import (_bass_exec_p, install_neuronx_cc_hook,
                                    partition_id_tensor)

    nc = _get_program(apply_qn)
    install_neuronx_cc_hook()

    partition_name = (nc.partition_id_tensor.name
                      if nc.partition_id_tensor else None)
    in_names, out_names, out_avals = [], [], []
    for alloc in nc.m.functions[0].allocations:
        if not isinstance(alloc, mybir.MemoryLocationSet):
            continue
        name = alloc.memorylocations[0].name
        if alloc.kind == "ExternalInput":
            if name != partition_name:
                in_names.append(name)
        elif alloc.kind == "ExternalOutput":
            shape = tuple(alloc.tensor_shape)
            dtype = mybir.dt.np(alloc.dtype)
            out_names.append(name)
            out_avals.append(jax.core.ShapedArray(shape, dtype))
    n_params = len(in_names)
    n_outs = len(out_names)
    all_in_names = in_names + out_names
    if partition_name is not None:
        all_in_names = all_in_names + [partition_name]
    donate = tuple(range(n_params, n_params + n_outs))

    def _body(*args):
        operands = list(args)
        if partition_name is not None:
            operands.append(partition_id_tensor())
        outs = _bass_exec_p.bind(
            *operands,
            out_avals=tuple(out_avals),
            in_names=tuple(all_in_names),
            out_names=tuple(out_names),
            lowering_input_output_aliases=(),
            sim_require_finite=True,
            sim_require_nnan=True,
            nc=nc,
        )
        return tuple(outs)

    devices = jax.devices()[:N_CORES]
    mesh = Mesh(np.asarray(devices), ("core",))
    in_specs = (PartitionSpec("core"),) * (n_params + n_outs)
    out_specs = (PartitionSpec("core"),) * n_outs
    fn = jax.jit(
        shard_map(_body, mesh=mesh, in_specs=in_specs, out_specs=out_specs,
                  check_rep=False),
        donate_argnums=donate, keep_unused=True)

    import jax.numpy as jnp
    from jax.sharding import NamedSharding
    zero_shardings = [NamedSharding(mesh, PartitionSpec("core"))] * n_outs
    zero_shapes = [(N_CORES * a.shape[0], *a.shape[1:]) for a in out_avals]
    zero_dtypes = [a.dtype for a in out_avals]

    def make_zeros():
        return [jax.device_put(jnp.zeros(s, d), sh)
                for s, d, sh in zip(zero_shapes, zero_dtypes, zero_shardings)]

    runner = {
        "fn": fn, "in_names": in_names, "out_names": out_names,
        "out_avals": out_avals, "make_zeros": make_zeros, "mesh": mesh,
    }
    _cache[key] = runner
    return runner


def _concat_inputs(runner, in_maps):
    return [np.concatenate([in_maps[c][name] for c in range(N_CORES)], axis=0)
            for name in runner["in_names"]]


def _run(runner, concat_in):
    out_arrs = runner["fn"](*concat_in, *runner["make_zeros"]())
    res = []
    for c in range(N_CORES):
        res.append({
            name: np.asarray(out_arrs[i]).reshape(
                N_CORES, *runner["out_avals"][i].shape)[c]
            for i, name in enumerate(runner["out_names"])})
    return res


def kernel(x, Wq, Wk, Wv, Wo, qn_w, kn_w):
    x = np.asarray(x, dtype=np.float32)
    Wq = np.asarray(Wq, dtype=np.float32)
    Wk = np.asarray(Wk, dtype=np.float32)
    Wv = np.asarray(Wv, dtype=np.float32)
    Wo = np.asarray(Wo, dtype=np.float32)
    qn_w = np.asarray(qn_w, dtype=np.float32)
    kn_w = np.asarray(kn_w, dtype=np.float32)

    apply_qn = not (np.all(qn_w == 1.0) and np.all(kn_w == 1.0))
    runner = _get_runner(apply_qn)
    in_maps = make_in_maps(x, Wq, Wk, Wv, Wo, qn_w, kn_w)
    res = _run(runner, _concat_inputs(runner, in_maps))
    return assemble_output(res)
